# revision 1
# baseline (speedup 1.0000x reference)
"""PhasorLayer TRN2 kernel: data-parallel over batch across 8 NeuronCores.

Math (per batch row m):
  u     = x @ [Wk|Wq|wsum]^T + [bk|bq|sum_bv]          (KQS gemm, N=129)
  align = 64 - 2*sum_p sin^2((pi/2)*(tanh(uk)-tanh(uq)))
  gain  = softplus(align/64 + 0.5);  s = align*gain/64
  w     = x @ Wv^T + bv        (V including bias)
  muw   = mean(w);  varw = mean(w^2) - muw^2
  inv   = rsqrt(s^2*varw + 1e-5);  a = s*inv;  c = a*muw
  out   = x + a*(w @ Wo'^T) - c*w1 + r
  where Wo' = Wo * ln_g (cols), w1 = rowsum(Wo'), r = ln_b @ Wo^T + bo
"""

import sys

sys.path.insert(0, "/opt/trn_rl_repo")

import math
import os
from contextlib import ExitStack

import numpy as np

import concourse.bass as bass
import concourse.mybir as mybir
import concourse.tile as tile
from concourse.alu_op_type import AluOpType
from concourse.bass_utils import run_bass_kernel_spmd
from concourse.mybir import dt
from concourse.tile_cfg import (
    BassTileBranchHintPlaceholder,
    BassTileConditionalBlock,
    BassTileCriticalSection,
    BassTileLoopBlock,
    BassTileSwitchBlock,
    TileBranchInst,
)
from concourse.vector_clock import ScopedClock

B, D, P = 8192, 4096, 64
NCORES = 8
M = B // NCORES  # 1024 batch rows per core
MT = M // 128    # 8 m-tiles
KD = D // 128    # 32 dim tiles
NB = D // 512    # 8 n-blocks
PI = math.pi
EPS = 1e-5
F32 = dt.float32
AF = mybir.ActivationFunctionType

USE_F32R = True  # big GEMMs in float32r (4x PE throughput); KQS stays fp32
MMDT = dt.float32r if USE_F32R else dt.float32

_SKIP_SPLIT = (
    BassTileBranchHintPlaceholder,
    BassTileConditionalBlock,
    BassTileCriticalSection,
    BassTileLoopBlock,
    BassTileSwitchBlock,
    TileBranchInst,
)


class LegalTileContext(tile.TileContext):
    """TileContext legalized to <=1 semaphore wait per instruction.

    This container's walrus rejects instructions with >1 sync wait. Extra
    waits are peeled onto single-wait NoOps on the same engine.
    """

    def _lower_ordered_insts(self, ordered):
        for insts in ordered.values():
            out = []
            for inst in insts:
                si = getattr(inst, "sync_info", None)
                if (
                    si is not None
                    and len(si.on_wait) > 1
                    and not isinstance(inst, _SKIP_SPLIT)
                ):
                    waits = list(si.on_wait)
                    for w in waits[:-1]:
                        nop = mybir.InstNoOp(
                            name=self.nc.get_next_instruction_name(),
                            text_hint="wait_split",
                            bass_nofuse=True,
                            engine=inst.engine,
                            sync_info=mybir.SyncInfo(on_wait=[w], on_update=[]),
                        )
                        out.append(nop)
                    inst.sync_info = mybir.SyncInfo(
                        on_wait=[waits[-1]], on_update=list(si.on_update)
                    )
                out.append(inst)
            insts[:] = out
        super()._lower_ordered_insts(ordered)

    def _drain_and_barrier(self, tick_clock, wait_clock):
        drain_inst = self.nc.sync.drain()
        wait_clock.add_sem_waits(
            drain_inst.ins, ScopedClock({None: tick_clock.global_clock})
        )
        si = drain_inst.ins.sync_info
        if si is not None and len(si.on_wait) > 1:
            waits = list(si.on_wait)
            drain_inst.ins.sync_info = mybir.SyncInfo(
                on_wait=[waits[0]], on_update=list(si.on_update)
            )
            for w in waits[1:]:
                nop = self.nc.sync.nop(nofuse=True, hint="wait_split")
                nop.ins.sync_info = mybir.SyncInfo(on_wait=[w], on_update=[])
        self.nc.all_engine_barrier()
        assert self.sems is not None
        popped = self.nc._tile_sem_poison_stack.pop()
        assert popped is self._sem_poison
        self.nc.clear_and_free_semaphores(list(self.sems.allocated().values()))
        self.nc.all_engine_barrier()


def _r(ap):
    return ap.bitcast(dt.float32r) if USE_F32R else ap


def build_nc(debug=False):
    nc = bass.Bass()
    x_d = nc.declare_dram_parameter("x", [M, D], F32, isOutput=False)
    xt_d = nc.declare_dram_parameter("xt", [D, M], MMDT, isOutput=False)
    wvt_d = nc.declare_dram_parameter("wvt", [D, D], MMDT, isOutput=False)
    wo2t_d = nc.declare_dram_parameter("wo2t", [D, D], MMDT, isOutput=False)
    wkqs_d = nc.declare_dram_parameter("wkqs", [D, 129], F32, isOutput=False)
    brow_d = nc.declare_dram_parameter("brow", [128, 129], F32, isOutput=False)
    bvr_d = nc.declare_dram_parameter("bvr", [128, KD], F32, isOutput=False)
    w1m_d = nc.declare_dram_parameter("w1m", [128, D], F32, isOutput=False)
    rm_d = nc.declare_dram_parameter("rm", [128, D], F32, isOutput=False)
    out_d = nc.declare_dram_parameter("out", [M, D], F32, isOutput=True)
    dbg_d = (
        nc.declare_dram_parameter("dbg", [128, 8 * MT + 129 * MT], F32, isOutput=True)
        if debug
        else None
    )

    wt_dram = nc.dram_tensor("wt_scr", [KD, 128, M], MMDT)
    ssq_dram = nc.dram_tensor("ssq_scr", [1, M], F32)

    with ExitStack() as ctx:
        tc = ctx.enter_context(LegalTileContext(nc))
        sb_small = ctx.enter_context(tc.tile_pool(name="small", bufs=1))

        ones_t = sb_small.tile((128, 1), F32, name="ones", tag="ones")
        nc.vector.memset(ones_t[:], 1.0)
        half_t = sb_small.tile((128, 1), F32, name="half", tag="half")
        nc.vector.memset(half_t[:], 0.5)
        eps_t = sb_small.tile((128, 1), F32, name="epsb", tag="epsb")
        nc.vector.memset(eps_t[:], EPS)
        brow_t = sb_small.tile((128, 129), F32, name="browt", tag="browt")
        nc.sync.dma_start(brow_t[:], brow_d[:, :])
        bvr_t = sb_small.tile((128, KD), F32, name="bvrt", tag="bvrt")
        nc.sync.dma_start(bvr_t[:], bvr_d[:, :])

        def col_tile(nm):
            return sb_small.tile((128, MT), F32, name=nm, tag=nm)

        red_all = col_tile("red_all")
        align_all = col_tile("align_all")
        e1_all = col_tile("e1_all")
        gain_all = col_tile("gain_all")
        s2_all = col_tile("s2_all")
        mu_all = col_tile("mu_all")
        ssq_all = col_tile("ssq_all")
        musq_all = col_tile("musq_all")
        var_all = col_tile("var_all")
        s_all = col_tile("s_all")
        s_sq_all = col_tile("s_sq_all")
        q_all = col_tile("q_all")
        q2_all = col_tile("q2_all")
        inv_all = col_tile("inv_all")
        a_all = col_tile("a_all")
        c_all = col_tile("c_all")
        cneg_all = col_tile("cneg_all")
        acc_sb = sb_small.tile((1, M), F32, name="acc_sb", tag="acc_sb")

        # ---------------- phase 1: KQS + GEMM1 (xt resident) ----------------
        with ExitStack() as p1:
            sb_xt = p1.enter_context(tc.tile_pool(name="xtp", bufs=1))
            sb_s1 = p1.enter_context(tc.tile_pool(name="s1", bufs=2))

            # f32r residents for GEMM1 are filled on-chip (ACT copy) from the
            # fp32 KQS stream below — x^T is read from DRAM exactly once.
            xt_ts = []
            for j in range(KD):
                t = sb_xt.tile((128, M), MMDT, name=f"xt{j}", tag=f"xt{j}")
                xt_ts.append(t)
            wkq_ts = []
            for j in range(KD):
                t = sb_xt.tile((128, 129), F32, name=f"wkq{j}", tag=f"wkq{j}")
                nc.sync.dma_start(t[:], wkqs_d[j * 128 : (j + 1) * 128, :])
                wkq_ts.append(t)

            # KQS gemm in true fp32: PE precision follows the operand's
            # memory dtype, so the f32r residents can't be used — stream
            # fp32-typed x^T tiles and accumulate all 8 m-tiles in PSUM.
            with ExitStack() as pk:
                ps_kq = pk.enter_context(
                    tc.tile_pool(name="pskq", bufs=1, space="PSUM")
                )
                kq_list = [
                    ps_kq.tile((128, 129), F32, name=f"kq{t}", tag=f"kq{t}")
                    for t in range(MT)
                ]
                for j in range(KD):
                    xs_t = sb_s1.tile(
                        (128, M), F32, name="xs_t", tag="xs", bufs=3
                    )
                    nc.scalar.dma_start(
                        xs_t[:], xt_d[j * 128 : (j + 1) * 128, :].bitcast(F32)
                    )
                    nc.scalar.activation(xt_ts[j][:], xs_t[:], AF.Copy)
                    for t in range(MT):
                        nc.tensor.matmul(
                            kq_list[t][:],
                            xs_t[:, t * 128 : (t + 1) * 128],
                            wkq_ts[j][:],
                            start=(j == 0),
                            stop=(j == KD - 1),
                        )
                for t in range(MT):
                    u_t = sb_s1.tile((128, 129), F32, name="u_t", tag="u")
                    nc.vector.tensor_add(u_t[:], kq_list[t][:], brow_t[:])
                    th_t = sb_s1.tile((128, 128), F32, name="th_t", tag="th")
                    nc.scalar.activation(th_t[:], u_t[:, 0:128], AF.Tanh)
                    d_t = sb_s1.tile((128, 64), F32, name="d_t", tag="d")
                    nc.vector.tensor_sub(d_t[:], th_t[:, 0:64], th_t[:, 64:128])
                    sn_t = sb_s1.tile((128, 64), F32, name="sn_t", tag="sn")
                    nc.scalar.activation(sn_t[:], d_t[:], AF.Sin, scale=PI / 2)
                    sq_t = sb_s1.tile((128, 64), F32, name="sq_t", tag="snsq")
                    nc.scalar.activation(
                        sq_t[:], sn_t[:], AF.Square, accum_out=red_all[:, t : t + 1]
                    )
                    nc.vector.tensor_scalar(
                        align_all[:, t : t + 1],
                        red_all[:, t : t + 1],
                        -2.0,
                        float(P),
                        AluOpType.mult,
                        AluOpType.add,
                    )
                    nc.scalar.activation(
                        e1_all[:, t : t + 1],
                        align_all[:, t : t + 1],
                        AF.Exp,
                        bias=half_t[:],
                        scale=1.0 / P,
                    )
                    nc.scalar.activation(
                        gain_all[:, t : t + 1], e1_all[:, t : t + 1], AF.Ln, bias=1.0
                    )
                    nc.vector.tensor_mul(
                        s2_all[:, t : t + 1],
                        align_all[:, t : t + 1],
                        gain_all[:, t : t + 1],
                    )
                    nc.scalar.activation(
                        mu_all[:, t : t + 1], u_t[:, 128:129], AF.Copy, scale=1.0 / D
                    )
                    if dbg_d is not None:
                        nc.sync.dma_start(
                            dbg_d[:, 8 * MT + t * 129 : 8 * MT + (t + 1) * 129],
                            u_t[:],
                        )

            # GEMM1: w^T = Wv @ x^T + bv. kd processed in pairs so Wv DMA rows
            # are 1KB (vs 512B). ssq = sum_k w^2 computed via sqw-stationary
            # matmuls straight into the [128, MT] per-row layout — no DRAM
            # transpose bounce.
            ps_v = p1.enter_context(tc.tile_pool(name="psv", bufs=1, space="PSUM"))
            ps_acc = p1.enter_context(
                tc.tile_pool(name="psacc", bufs=1, space="PSUM")
            )
            acc_ps0 = ps_acc.tile((1, 512), F32, name="acc_ps0", tag="acc0")
            acc_ps1 = ps_acc.tile((1, 512), F32, name="acc_ps1", tag="acc1")
            for kdg in range(KD // 2):
                v_ps = [
                    ps_v.tile((128, 512), F32, name=f"v_ps{i}", tag=f"v{i}")
                    for i in range(4)
                ]
                for j in range(KD):
                    wv_t = sb_s1.tile((128, 256), MMDT, name="wv_t", tag="wv", bufs=4)
                    wv_eng = (
                        nc.sync
                        if j % 4 in (0, 2)
                        else (nc.scalar if j % 4 == 1 else nc.gpsimd)
                    )
                    wv_eng.dma_start(
                        wv_t[:],
                        wvt_d[j * 128 : (j + 1) * 128, kdg * 256 : (kdg + 1) * 256],
                    )
                    for i in range(4):
                        sub, half = divmod(i, 2)
                        nc.tensor.matmul(
                            v_ps[i][:],
                            wv_t[:, sub * 128 : (sub + 1) * 128],
                            xt_ts[j][:, half * 512 : (half + 1) * 512],
                            start=(j == 0),
                            stop=(j == KD - 1),
                        )
                for sub in range(2):
                    kd = kdg * 2 + sub
                    wt_t = sb_s1.tile((128, M), F32, name="wt_t", tag="wt")
                    nc.vector.tensor_scalar(
                        wt_t[:, 0:512],
                        v_ps[sub * 2][:],
                        bvr_t[:, kd : kd + 1],
                        None,
                        AluOpType.add,
                    )
                    nc.vector.tensor_scalar(
                        wt_t[:, 512:1024],
                        v_ps[sub * 2 + 1][:],
                        bvr_t[:, kd : kd + 1],
                        None,
                        AluOpType.add,
                    )
                    sqw_t = sb_s1.tile((128, M), F32, name="sqw_t", tag="sqw")
                    nc.scalar.activation(sqw_t[:], wt_t[:], AF.Square)
                    nc.tensor.matmul(
                        acc_ps0[:],
                        ones_t[:],
                        sqw_t[:, 0:512],
                        start=(kd == 0),
                        stop=(kd == KD - 1),
                    )
                    nc.tensor.matmul(
                        acc_ps1[:],
                        ones_t[:],
                        sqw_t[:, 512:1024],
                        start=(kd == 0),
                        stop=(kd == KD - 1),
                    )
                    nc.gpsimd.dma_start(wt_dram[kd, :, :], _r(wt_t[:]))

            # ssq bounce: [1, M] -> DRAM -> [128, MT] columns
            nc.scalar.copy(acc_sb[:, 0:512], acc_ps0[:])
            nc.scalar.copy(acc_sb[:, 512:1024], acc_ps1[:])
            nc.sync.dma_start(ssq_dram[:, :], acc_sb[:])
            for t in range(MT):
                nc.sync.dma_start(
                    ssq_all[:, t : t + 1],
                    ssq_dram[0:1, t * 128 : (t + 1) * 128].transpose([1, 0]),
                )

        # ---------------- scalar finalize ----------------
        nc.scalar.activation(musq_all[:], mu_all[:], AF.Square)
        nc.vector.tensor_scalar(
            var_all[:], ssq_all[:], 1.0 / D, None, AluOpType.mult
        )
        nc.vector.tensor_sub(var_all[:], var_all[:], musq_all[:])
        nc.scalar.activation(s_all[:], s2_all[:], AF.Copy, scale=1.0 / P)
        nc.scalar.activation(s_sq_all[:], s_all[:], AF.Square)
        nc.vector.tensor_mul(q_all[:], var_all[:], s_sq_all[:])
        nc.scalar.activation(q2_all[:], q_all[:], AF.Sqrt, bias=eps_t[:])
        nc.vector.reciprocal(inv_all[:], q2_all[:])
        nc.vector.tensor_mul(a_all[:], s_all[:], inv_all[:])
        nc.vector.tensor_mul(c_all[:], a_all[:], mu_all[:])
        nc.vector.tensor_scalar(
            cneg_all[:], c_all[:], -1.0, None, AluOpType.mult
        )
        if dbg_d is not None:
            for i, dt_ in enumerate(
                [red_all, align_all, mu_all, ssq_all, var_all, s_all, a_all, c_all]
            ):
                nc.sync.dma_start(dbg_d[:, i * MT : (i + 1) * MT], dt_[:])

        # ---------------- phase 2: GEMM2 + epilogue (wt resident) ----------------
        with ExitStack() as p2:
            sb_wt = p2.enter_context(tc.tile_pool(name="wtp", bufs=1))
            sb_s2 = p2.enter_context(tc.tile_pool(name="s2", bufs=2))
            ps_p = p2.enter_context(tc.tile_pool(name="psp", bufs=1, space="PSUM"))

            wt_res = []
            for k in range(KD):
                t = sb_wt.tile((128, M), MMDT, name=f"wtr{k}", tag=f"wtr{k}")
                nc.gpsimd.dma_start(t[:], wt_dram[k, :, :])
                wt_res.append(t)
            w1_res = sb_wt.tile((128, D), F32, name="w1_res", tag="w1_res")
            nc.sync.dma_start(w1_res[:], w1m_d[:, :])
            r_res = sb_wt.tile((128, D), F32, name="r_res", tag="r_res")
            nc.sync.dma_start(r_res[:], rm_d[:, :])

            for nb in range(NB):
                nsl = slice(nb * 512, (nb + 1) * 512)
                p_tiles = [
                    ps_p.tile((128, 512), F32, name=f"pp{mt}", tag=f"pp{mt}")
                    for mt in range(MT)
                ]
                for k in range(KD):
                    wo_t = sb_s2.tile((128, 512), MMDT, name="wo_t", tag="wo", bufs=3)
                    wo_eng = (nc.sync, nc.scalar, nc.gpsimd)[k % 3]
                    wo_eng.dma_start(wo_t[:], wo2t_d[k * 128 : (k + 1) * 128, nsl])
                    for mt in range(MT):
                        nc.tensor.matmul(
                            p_tiles[mt][:],
                            wt_res[k][:, mt * 128 : (mt + 1) * 128],
                            wo_t[:],
                            start=(k == 0),
                            stop=(k == KD - 1),
                        )
                for mt in range(MT):
                    msl = slice(mt * 128, (mt + 1) * 128)
                    xe_t = sb_s2.tile((128, 512), F32, name="xe_t", tag="xe")
                    nc.scalar.dma_start(xe_t[:], x_d[msl, nsl])
                    t1_t = sb_s2.tile((128, 512), F32, name="t1_t", tag="t1")
                    nc.vector.scalar_tensor_tensor(
                        t1_t[:],
                        p_tiles[mt][:],
                        a_all[:, mt : mt + 1],
                        xe_t[:],
                        AluOpType.mult,
                        AluOpType.add,
                    )
                    u2_t = sb_s2.tile((128, 512), F32, name="u2_t", tag="u2")
                    nc.vector.scalar_tensor_tensor(
                        u2_t[:],
                        w1_res[:, nsl],
                        cneg_all[:, mt : mt + 1],
                        r_res[:, nsl],
                        AluOpType.mult,
                        AluOpType.add,
                    )
                    oe_t = sb_s2.tile((128, 512), F32, name="oe_t", tag="oe")
                    nc.vector.tensor_add(oe_t[:], t1_t[:], u2_t[:])
                    nc.sync.dma_start(out_d[msl, nsl], oe_t[:])
    return nc


def kernel(**inputs):
    x = np.asarray(inputs["x"], dtype=np.float32)
    Wk = np.asarray(inputs["Wk"], dtype=np.float32)
    bk = np.asarray(inputs["bk"], dtype=np.float32)
    Wq = np.asarray(inputs["Wq"], dtype=np.float32)
    bq = np.asarray(inputs["bq"], dtype=np.float32)
    Wv = np.asarray(inputs["Wv"], dtype=np.float32)
    bv = np.asarray(inputs["bv"], dtype=np.float32)
    ln_g = np.asarray(inputs["ln_g"], dtype=np.float32)
    ln_b = np.asarray(inputs["ln_b"], dtype=np.float32)
    Wo = np.asarray(inputs["Wo"], dtype=np.float32)
    bo = np.asarray(inputs["bo"], dtype=np.float32)

    Wo2T = np.ascontiguousarray((Wo * ln_g[None, :]).T)  # [k, n] = Wo'[n, k]
    w1 = Wo2T.sum(axis=0)  # [n]
    r = ln_b @ Wo.T + bo  # [n]
    WvT = np.ascontiguousarray(Wv.T)  # [j, k]
    wsum = Wv.sum(axis=0)  # [j]
    wkqs = np.ascontiguousarray(
        np.concatenate([Wk.T, Wq.T, wsum[:, None]], axis=1), dtype=np.float32
    )  # [D, 129]
    brow = np.concatenate([bk, bq, [bv.sum()]]).astype(np.float32)  # [129]
    brow_mat = np.ascontiguousarray(np.broadcast_to(brow, (128, 129)))
    w1_mat = np.ascontiguousarray(np.broadcast_to(w1, (128, D)), dtype=np.float32)
    r_mat = np.ascontiguousarray(np.broadcast_to(r, (128, D)), dtype=np.float32)
    bvr = np.ascontiguousarray(bv.reshape(KD, 128).T)  # [128, KD]

    debug = os.environ.get("PHASOR_DEBUG") == "1"
    nc = build_nc(debug=debug)
    in_maps = []
    for c in range(NCORES):
        xc = np.ascontiguousarray(x[c * M : (c + 1) * M])
        xtc = np.ascontiguousarray(xc.T)
        in_maps.append(
            {
                "x": xc,
                "xt": xtc,
                "wvt": WvT,
                "wo2t": Wo2T,
                "wkqs": wkqs,
                "brow": brow_mat,
                "bvr": bvr,
                "w1m": w1_mat,
                "rm": r_mat,
            }
        )
    global LAST_BUILD
    LAST_BUILD = (nc, in_maps)
    trace = os.environ.get("PHASOR_TRACE") == "1"
    res = run_bass_kernel_spmd(
        nc, in_maps, core_ids=list(range(NCORES)), trace=trace
    )
    global LAST_EXEC_NS
    LAST_EXEC_NS = getattr(res, "exec_time_ns", None)
    if debug:
        global LAST_DBG
        LAST_DBG = [res.results[c]["dbg"] for c in range(NCORES)]
    out = np.concatenate([res.results[c]["out"] for c in range(NCORES)], axis=0)
    return out.astype(np.float32)


LAST_EXEC_NS = None
LAST_BUILD = None
LAST_DBG = None


if __name__ == "__main__":
    rng = np.random.default_rng(0)
    ins = {
        "x": rng.standard_normal((B, D), dtype=np.float32),
        "Wk": rng.standard_normal((P, D), dtype=np.float32) / math.sqrt(D),
        "bk": np.zeros(P, np.float32),
        "Wq": rng.standard_normal((P, D), dtype=np.float32) / math.sqrt(D),
        "bq": np.zeros(P, np.float32),
        "Wv": rng.standard_normal((D, D), dtype=np.float32) / math.sqrt(D),
        "bv": np.zeros(D, np.float32),
        "ln_g": np.ones(D, np.float32),
        "ln_b": np.zeros(D, np.float32),
        "Wo": rng.standard_normal((D, D), dtype=np.float32) / math.sqrt(D),
        "bo": np.zeros(D, np.float32),
    }
    out = kernel(**ins)
    print("out", out.shape, out.dtype, float(np.abs(out).mean()))



# revision 2
# speedup vs baseline: 4.3863x; 4.3863x over previous
"""PhasorLayer TRN2 kernel v2: data-parallel over batch, 8 NeuronCores.

Single-pass bf16 design (per batch row m):
  u     = x @ [Wk|Wq|wsum]^T + [bk|bq|sum_bv]          (KQS gemm, N=129, bf16)
  align = 64 - 2*sum_p sin^2((pi/2)*(tanh(uk)-tanh(uq)))
  gain  = softplus(align/64 + 0.5);  s = align*gain/64
  w     = x @ Wv^T + bv        (GEMM1, bf16, V^T kept resident in SBUF)
  muw   = mean(w);  varw = mean(w^2) - muw^2
  inv   = rsqrt(s^2*varw + 1e-5);  a = s*inv;  c = a*muw
  out   = xr + a*(w @ Wo'^T) - c*w1                    (GEMM2, bf16)
  where Wo' = Wo * ln_g (cols), w1 = rowsum(Wo'), xr = x + ln_b@Wo^T + bo
"""

import sys

sys.path.insert(0, "/opt/trn_rl_repo")

import math
import os
from contextlib import ExitStack

import ml_dtypes
import numpy as np

import concourse.bass as bass
import concourse.mybir as mybir
import concourse.tile as tile
from concourse.alu_op_type import AluOpType
from concourse.bass_utils import run_bass_kernel_spmd
from concourse.mybir import dt
from concourse.tile_cfg import (
    BassTileBranchHintPlaceholder,
    BassTileConditionalBlock,
    BassTileCriticalSection,
    BassTileLoopBlock,
    BassTileSwitchBlock,
    TileBranchInst,
)
from concourse.vector_clock import ScopedClock

B, D, P = 8192, 4096, 64
NCORES = 8
M = B // NCORES  # 1024 batch rows per core
MT = M // 128    # 8 m-tiles
KD = D // 128    # 32 dim tiles
NB = D // 512    # 8 n-blocks
PI = math.pi
EPS = 1e-5
F32 = dt.float32
BF16 = dt.bfloat16
NPBF16 = ml_dtypes.bfloat16
AF = mybir.ActivationFunctionType

_SKIP_SPLIT = (
    BassTileBranchHintPlaceholder,
    BassTileConditionalBlock,
    BassTileCriticalSection,
    BassTileLoopBlock,
    BassTileSwitchBlock,
    TileBranchInst,
)


class LegalTileContext(tile.TileContext):
    """TileContext legalized to <=1 semaphore wait per instruction.

    This container's walrus rejects instructions with >1 sync wait. Extra
    waits are peeled onto single-wait NoOps on the same engine.
    """

    def _lower_ordered_insts(self, ordered):
        for insts in ordered.values():
            out = []
            for inst in insts:
                si = getattr(inst, "sync_info", None)
                if (
                    si is not None
                    and len(si.on_wait) > 1
                    and not isinstance(inst, _SKIP_SPLIT)
                ):
                    waits = list(si.on_wait)
                    for w in waits[:-1]:
                        nop = mybir.InstNoOp(
                            name=self.nc.get_next_instruction_name(),
                            text_hint="wait_split",
                            bass_nofuse=True,
                            engine=inst.engine,
                            sync_info=mybir.SyncInfo(on_wait=[w], on_update=[]),
                        )
                        out.append(nop)
                    inst.sync_info = mybir.SyncInfo(
                        on_wait=[waits[-1]], on_update=list(si.on_update)
                    )
                out.append(inst)
            insts[:] = out
        super()._lower_ordered_insts(ordered)

    def _drain_and_barrier(self, tick_clock, wait_clock):
        drain_inst = self.nc.sync.drain()
        wait_clock.add_sem_waits(
            drain_inst.ins, ScopedClock({None: tick_clock.global_clock})
        )
        si = drain_inst.ins.sync_info
        if si is not None and len(si.on_wait) > 1:
            waits = list(si.on_wait)
            drain_inst.ins.sync_info = mybir.SyncInfo(
                on_wait=[waits[0]], on_update=list(si.on_update)
            )
            for w in waits[1:]:
                nop = self.nc.sync.nop(nofuse=True, hint="wait_split")
                nop.ins.sync_info = mybir.SyncInfo(on_wait=[w], on_update=[])
        self.nc.all_engine_barrier()
        assert self.sems is not None
        popped = self.nc._tile_sem_poison_stack.pop()
        assert popped is self._sem_poison
        self.nc.clear_and_free_semaphores(list(self.sems.allocated().values()))
        self.nc.all_engine_barrier()


def build_body(nc, tc, ctx, dram, rep):
    """Emit one full kernel execution under TileContext tc."""
    r = f"r{rep}_"
    (xt_d, wvb_d, wob_d, wkqsb_d, brow_d, bvr_d, w1m_d, xrm_d, out_d,
     ssq_dram) = dram

    sb_small = ctx.enter_context(tc.tile_pool(name=r + "small", bufs=1))

    ones_t = sb_small.tile((128, 1), BF16, name=r + "ones", tag=r + "ones")
    nc.vector.memset(ones_t[:], 1.0)
    half_t = sb_small.tile((128, 1), F32, name=r + "half", tag=r + "half")
    nc.vector.memset(half_t[:], 0.5)
    eps_t = sb_small.tile((128, 1), F32, name=r + "epsb", tag=r + "epsb")
    nc.vector.memset(eps_t[:], EPS)
    brow_t = sb_small.tile((128, 129), F32, name=r + "browt", tag=r + "browt")
    nc.sync.dma_start(brow_t[:], brow_d[:, :])
    bvr_t = sb_small.tile((128, KD), F32, name=r + "bvrt", tag=r + "bvrt")
    nc.sync.dma_start(bvr_t[:], bvr_d[:, :])

    def col_tile(nm):
        return sb_small.tile((128, MT), F32, name=r + nm, tag=r + nm)

    red_all = col_tile("red_all")
    align_all = col_tile("align_all")
    e1_all = col_tile("e1_all")
    gain_all = col_tile("gain_all")
    s2_all = col_tile("s2_all")
    mu_all = col_tile("mu_all")
    ssq_all = col_tile("ssq_all")
    musq_all = col_tile("musq_all")
    var_all = col_tile("var_all")
    s_all = col_tile("s_all")
    s_sq_all = col_tile("s_sq_all")
    q_all = col_tile("q_all")
    q2_all = col_tile("q2_all")
    inv_all = col_tile("inv_all")
    a_all = col_tile("a_all")
    c_all = col_tile("c_all")
    cneg_all = col_tile("cneg_all")
    acc_sb = sb_small.tile((1, M), F32, name=r + "acc_sb", tag=r + "acc_sb")

    # wt residents persist across both phases
    sb_wt = ctx.enter_context(tc.tile_pool(name=r + "wtp", bufs=1))
    wt_res = [
        sb_wt.tile((128, M), BF16, name=f"{r}wtr{k}", tag=f"{r}wtr{k}")
        for k in range(KD)
    ]
    # prefetch buffer for GEMM2 nb=0 kd 0..7 — no deps, loads at t=0
    wo_pre = sb_wt.tile((128, 8 * 512), BF16, name=r + "wo_pre", tag=r + "wo_pre")
    nc.scalar.dma_start(wo_pre[:], wob_d[:, 0 : 8 * 512])

    # ---------------- phase 1: KQS + GEMM1 (xt resident) ----------------
    with ExitStack() as p1:
        sb_xt = p1.enter_context(tc.tile_pool(name=r + "xtp", bufs=1))
        sb_s1 = p1.enter_context(tc.tile_pool(name=r + "s1", bufs=2))

        # bf16 x^T residents for GEMM1, produced on-chip from the fp32 KQS
        # stream (the phase path needs full fp32 accuracy: align is centered
        # near 0 and a = s*rsqrt(s^2 var + eps) amplifies bf16 noise ~300x
        # on near-zero-resonance rows).
        xt_res = [
            sb_xt.tile((128, M), BF16, name=f"{r}xt{j}", tag=f"{r}xt{j}")
            for j in range(KD)
        ]

        def xt_j(j):
            return xt_res[j][:]

        wkq_t = sb_xt.tile((128, KD * 129), F32, name=r + "wkq", tag=r + "wkq")
        nc.scalar.dma_start(wkq_t[:], wkqsb_d[:, :])

        # KQS gemm in fp32: stationary = x^T m-slice, moving = wkqs[j]
        with ExitStack() as pk:
            ps_kq = pk.enter_context(tc.tile_pool(name=r + "pskq", bufs=1, space="PSUM"))
            kq_list = [
                ps_kq.tile((128, 129), F32, name=f"{r}kq{t}", tag=f"{r}kq{t}")
                for t in range(MT)
            ]
            for j in range(KD):
                xs_t = sb_s1.tile((128, M), F32, name=r + "xs_t", tag=r + "xs", bufs=3)
                xs_eng = (nc.sync, nc.scalar, nc.gpsimd)[j % 3]
                xs_eng.dma_start(xs_t[:], xt_d[j * 128 : (j + 1) * 128, :])
                nc.scalar.activation(xt_res[j][:], xs_t[:], AF.Copy)
                for t in range(MT):
                    nc.tensor.matmul(
                        kq_list[t][:],
                        xs_t[:, t * 128 : (t + 1) * 128],
                        wkq_t[:, j * 129 : (j + 1) * 129],
                        start=(j == 0),
                        stop=(j == KD - 1),
                    )
            # free all kq PSUM banks promptly: copy u = kq + brow to SBUF
            u_ts = []
            for t in range(MT):
                u_t = sb_s1.tile((128, 129), F32, name=r + "u_t", tag=r + "u", bufs=8)
                nc.vector.tensor_add(u_t[:], kq_list[t][:], brow_t[:])
                u_ts.append(u_t)

        # per-m-tile phase epilogue (ACT/DVE; overlaps GEMM1 matmuls on PE)
        for t in range(MT):
            u_t = u_ts[t]
            th_t = sb_s1.tile((128, 128), F32, name=r + "th_t", tag=r + "th")
            nc.scalar.activation(th_t[:], u_t[:, 0:128], AF.Tanh)
            d_t = sb_s1.tile((128, 64), F32, name=r + "d_t", tag=r + "d")
            nc.vector.tensor_sub(d_t[:], th_t[:, 0:64], th_t[:, 64:128])
            sn_t = sb_s1.tile((128, 64), F32, name=r + "sn_t", tag=r + "sn")
            nc.scalar.activation(sn_t[:], d_t[:], AF.Sin, scale=PI / 2)
            sq_t = sb_s1.tile((128, 64), F32, name=r + "sq_t", tag=r + "snsq")
            nc.scalar.activation(
                sq_t[:], sn_t[:], AF.Square, accum_out=red_all[:, t : t + 1]
            )
            nc.vector.tensor_scalar(
                align_all[:, t : t + 1],
                red_all[:, t : t + 1],
                -2.0,
                float(P),
                AluOpType.mult,
                AluOpType.add,
            )
            nc.scalar.activation(
                e1_all[:, t : t + 1],
                align_all[:, t : t + 1],
                AF.Exp,
                bias=half_t[:],
                scale=1.0 / P,
            )
            nc.scalar.activation(
                gain_all[:, t : t + 1], e1_all[:, t : t + 1], AF.Ln, bias=1.0
            )
            nc.vector.tensor_mul(
                s2_all[:, t : t + 1],
                align_all[:, t : t + 1],
                gain_all[:, t : t + 1],
            )
            nc.scalar.activation(
                mu_all[:, t : t + 1], u_t[:, 128:129], AF.Copy, scale=1.0 / D
            )

        # GEMM1: w^T tile kd = sum_j Wv^T[j, kd]^T @ x^T[j]  -> SBUF bf16
        # ssq = sum_k w^2 via ones-stationary matmuls, staggered one kd
        # behind the main stream so the PE never waits on sqw.
        ps_v = p1.enter_context(tc.tile_pool(name=r + "psv", bufs=2, space="PSUM"))
        ps_acc = p1.enter_context(tc.tile_pool(name=r + "psacc", bufs=1, space="PSUM"))
        acc_ps0 = ps_acc.tile((1, 512), F32, name=r + "acc_ps0", tag=r + "acc0")
        acc_ps1 = ps_acc.tile((1, 512), F32, name=r + "acc_ps1", tag=r + "acc1")

        sqw_tiles = [None] * KD

        def emit_ssq(kd):
            sqw_t = sqw_tiles[kd]
            nc.tensor.matmul(
                acc_ps0[:], ones_t[:], sqw_t[:, 0:512],
                start=(kd == 0), stop=(kd == KD - 1),
            )
            nc.tensor.matmul(
                acc_ps1[:], ones_t[:], sqw_t[:, 512:1024],
                start=(kd == 0), stop=(kd == KD - 1),
            )

        for kd in range(KD):
            wv_t = sb_s1.tile((128, D), BF16, name=r + "wv_t", tag=r + "wv", bufs=3)
            wv_eng = (nc.sync, nc.scalar, nc.gpsimd)[kd % 3]
            wv_eng.dma_start(wv_t[:], wvb_d[:, kd * D : (kd + 1) * D])
            v_ps = [
                ps_v.tile((128, 512), F32, name=f"{r}v_ps{h}", tag=f"{r}vps{h}")
                for h in range(2)
            ]
            for b in range(KD):
                for h in range(2):
                    nc.tensor.matmul(
                        v_ps[h][:],
                        wv_t[:, b * 128 : (b + 1) * 128],
                        xt_j(b)[:, h * 512 : (h + 1) * 512],
                        start=(b == 0),
                        stop=(b == KD - 1),
                    )
            if kd >= 1:
                emit_ssq(kd - 1)
            # bias add + cast to bf16 resident; square for ssq
            for h in range(2):
                nc.vector.tensor_scalar(
                    wt_res[kd][:, h * 512 : (h + 1) * 512],
                    v_ps[h][:],
                    bvr_t[:, kd : kd + 1],
                    None,
                    AluOpType.add,
                )
            sqw_t = sb_s1.tile((128, M), BF16, name=r + "sqw_t", tag=r + "sqw", bufs=2)
            nc.scalar.activation(sqw_t[:], wt_res[kd][:], AF.Square)
            sqw_tiles[kd] = sqw_t
        emit_ssq(KD - 1)

        # ssq bounce: [1, M] -> DRAM -> [128, MT] columns
        nc.scalar.copy(acc_sb[:, 0:512], acc_ps0[:])
        nc.scalar.copy(acc_sb[:, 512:1024], acc_ps1[:])
        nc.sync.dma_start(ssq_dram[:, :], acc_sb[:])
        for t in range(MT):
            nc.sync.dma_start(
                ssq_all[:, t : t + 1],
                ssq_dram[0:1, t * 128 : (t + 1) * 128].transpose([1, 0]),
            )

    # ---------------- scalar finalize ----------------
    nc.scalar.activation(musq_all[:], mu_all[:], AF.Square)
    nc.vector.tensor_scalar(var_all[:], ssq_all[:], 1.0 / D, None, AluOpType.mult)
    nc.vector.tensor_sub(var_all[:], var_all[:], musq_all[:])
    nc.scalar.activation(s_all[:], s2_all[:], AF.Copy, scale=1.0 / P)
    nc.scalar.activation(s_sq_all[:], s_all[:], AF.Square)
    nc.vector.tensor_mul(q_all[:], var_all[:], s_sq_all[:])
    nc.scalar.activation(q2_all[:], q_all[:], AF.Sqrt, bias=eps_t[:])
    nc.vector.reciprocal(inv_all[:], q2_all[:])
    nc.vector.tensor_mul(a_all[:], s_all[:], inv_all[:])
    nc.vector.tensor_mul(c_all[:], a_all[:], mu_all[:])
    nc.vector.tensor_scalar(cneg_all[:], c_all[:], -1.0, None, AluOpType.mult)

    # ---------------- phase 2: GEMM2 + epilogue (wt resident) ----------------
    with ExitStack() as p2:
        sb_s2 = p2.enter_context(tc.tile_pool(name=r + "s2", bufs=2))
        ps_p = p2.enter_context(tc.tile_pool(name=r + "psp", bufs=1, space="PSUM"))

        w1_res = sb_s2.tile((128, D), F32, name=r + "w1_res", tag=r + "w1_res", bufs=1)
        nc.sync.dma_start(w1_res[:], w1m_d[:, :])

        QN = 8 * 512  # wo quarter: 8 kd-slices of one nb

        def epilogue(nb, mt, p_tile):
            nsl = slice(nb * 512, (nb + 1) * 512)
            msl = slice(mt * 128, (mt + 1) * 128)
            xe_t = sb_s2.tile((128, 512), F32, name=r + "xe_t", tag=r + "xe", bufs=3)
            nc.scalar.dma_start(xe_t[:], xrm_d[msl, nsl])
            t1_t = sb_s2.tile((128, 512), F32, name=r + "t1_t", tag=r + "t1", bufs=3)
            nc.vector.scalar_tensor_tensor(
                t1_t[:], p_tile[:], a_all[:, mt : mt + 1], xe_t[:],
                AluOpType.mult, AluOpType.add,
            )
            oe_t = sb_s2.tile((128, 512), F32, name=r + "oe_t", tag=r + "oe", bufs=3)
            nc.vector.scalar_tensor_tensor(
                oe_t[:], w1_res[:, nsl], cneg_all[:, mt : mt + 1], t1_t[:],
                AluOpType.mult, AluOpType.add,
            )
            nc.sync.dma_start(out_d[msl, nsl], oe_t[:])

        for nb in range(NB):
            # wo quarters for this nb (kd 0-7 of nb=0 comes from wo_pre)
            wo_q = []
            for q in range(4):
                if nb == 0 and q == 0:
                    wo_q.append(wo_pre)
                    continue
                t = sb_s2.tile((128, QN), BF16, name=r + "wo_q", tag=r + "woq", bufs=5)
                eng = (nc.gpsimd, nc.scalar)[(nb * 4 + q) % 2]
                eng.dma_start(
                    t[:],
                    wob_d[:, nb * (KD * 512) + q * QN : nb * (KD * 512) + (q + 1) * QN],
                )
                wo_q.append(t)

            p_tiles = [
                ps_p.tile((128, 512), F32, name=f"{r}pp{mt}", tag=f"{r}pp{mt}")
                for mt in range(MT)
            ]
            for half in range(2):
                mts = range(half * 4, half * 4 + 4)
                for kd in range(KD):
                    wo_sl = wo_q[kd // 8][:, (kd % 8) * 512 : (kd % 8 + 1) * 512]
                    for mt in mts:
                        nc.tensor.matmul(
                            p_tiles[mt][:],
                            wt_res[kd][:, mt * 128 : (mt + 1) * 128],
                            wo_sl,
                            start=(kd == 0),
                            stop=(kd == KD - 1),
                        )
                for mt in mts:
                    epilogue(nb, mt, p_tiles[mt])


def build_nc(reps=1):
    nc = bass.Bass()
    xt_d = nc.declare_dram_parameter("xt", [D, M], F32, isOutput=False)
    wvb_d = nc.declare_dram_parameter("wvb", [128, KD * D], BF16, isOutput=False)
    wob_d = nc.declare_dram_parameter("wob", [128, NB * KD * 512], BF16, isOutput=False)
    wkqsb_d = nc.declare_dram_parameter("wkqsb", [128, KD * 129], F32, isOutput=False)
    brow_d = nc.declare_dram_parameter("brow", [128, 129], F32, isOutput=False)
    bvr_d = nc.declare_dram_parameter("bvr", [128, KD], F32, isOutput=False)
    w1m_d = nc.declare_dram_parameter("w1m", [128, D], F32, isOutput=False)
    xrm_d = nc.declare_dram_parameter("xrm", [M, D], F32, isOutput=False)
    out_d = nc.declare_dram_parameter("out", [M, D], F32, isOutput=True)
    ssq_dram = nc.dram_tensor("ssq_scr", [1, M], F32)
    dram = (xt_d, wvb_d, wob_d, wkqsb_d, brow_d, bvr_d, w1m_d, xrm_d,
            out_d, ssq_dram)

    for rep in range(reps):
        with ExitStack() as ctx:
            tc = ctx.enter_context(LegalTileContext(nc))
            build_body(nc, tc, ctx, dram, rep)
    return nc


def prep_host(inputs):
    """Host-side weight layout prep shared by all cores."""
    Wk = np.asarray(inputs["Wk"], dtype=np.float32)
    bk = np.asarray(inputs["bk"], dtype=np.float32)
    Wq = np.asarray(inputs["Wq"], dtype=np.float32)
    bq = np.asarray(inputs["bq"], dtype=np.float32)
    Wv = np.asarray(inputs["Wv"], dtype=np.float32)
    bv = np.asarray(inputs["bv"], dtype=np.float32)
    ln_g = np.asarray(inputs["ln_g"], dtype=np.float32)
    ln_b = np.asarray(inputs["ln_b"], dtype=np.float32)
    Wo = np.asarray(inputs["Wo"], dtype=np.float32)
    bo = np.asarray(inputs["bo"], dtype=np.float32)

    Wo2T = np.ascontiguousarray((Wo * ln_g[None, :]).T)  # [k, n]
    w1 = Wo2T.sum(axis=0)  # [n]
    rrow = (ln_b @ Wo.T + bo).astype(np.float32)  # [n]
    wsum = Wv.sum(axis=0)  # [j]
    wkqs = np.concatenate([Wk.T, Wq.T, wsum[:, None]], axis=1).astype(np.float32)
    brow = np.concatenate([bk, bq, [bv.sum()]]).astype(np.float32)

    # [128, KD*D]: wvb[p, kd*D + b*128 + kk] = Wv[kd*128+kk, b*128+p]
    wvb = np.ascontiguousarray(
        Wv.reshape(KD, 128, KD, 128).transpose(3, 0, 2, 1).reshape(128, KD * D)
    ).astype(NPBF16)
    # [128, NB*KD*512]: wob[p, nb*KD*512 + kd*512 + n'] = Wo2T[kd*128+p, nb*512+n']
    wob = np.ascontiguousarray(
        Wo2T.reshape(KD, 128, NB, 512).transpose(1, 2, 0, 3).reshape(128, NB * KD * 512)
    ).astype(NPBF16)
    # [128, KD*129]: wkqsb[p, j*129+c] = wkqs[j*128+p, c]  (fp32)
    wkqsb = np.ascontiguousarray(
        wkqs.reshape(KD, 128, 129).transpose(1, 0, 2).reshape(128, KD * 129)
    )
    brow_mat = np.ascontiguousarray(np.broadcast_to(brow, (128, 129)))
    bvr = np.ascontiguousarray(bv.reshape(KD, 128).T)  # [128, KD]
    w1m = np.ascontiguousarray(np.broadcast_to(w1, (128, D)), dtype=np.float32)
    return wvb, wob, wkqsb, brow_mat, bvr, w1m, rrow


def make_in_maps(x, prep):
    wvb, wob, wkqsb, brow_mat, bvr, w1m, rrow = prep
    in_maps = []
    for c in range(NCORES):
        xc = np.ascontiguousarray(x[c * M : (c + 1) * M])
        xtc = np.ascontiguousarray(xc.T)
        xrm = np.ascontiguousarray(xc + rrow[None, :])
        in_maps.append(
            {
                "xt": xtc,
                "wvb": wvb,
                "wob": wob,
                "wkqsb": wkqsb,
                "brow": brow_mat,
                "bvr": bvr,
                "w1m": w1m,
                "xrm": xrm,
            }
        )
    return in_maps


def kernel(**inputs):
    x = np.asarray(inputs["x"], dtype=np.float32)
    prep = prep_host(inputs)
    nc = build_nc(reps=1)
    in_maps = make_in_maps(x, prep)
    global LAST_BUILD
    LAST_BUILD = (nc, in_maps)
    res = run_bass_kernel_spmd(nc, in_maps, core_ids=list(range(NCORES)))
    global LAST_EXEC_NS
    LAST_EXEC_NS = getattr(res, "exec_time_ns", None)
    out = np.concatenate([res.results[c]["out"] for c in range(NCORES)], axis=0)
    return out.astype(np.float32)


LAST_EXEC_NS = None
LAST_BUILD = None


if __name__ == "__main__":
    rng = np.random.default_rng(0)
    ins = {
        "x": rng.standard_normal((B, D), dtype=np.float32),
        "Wk": rng.standard_normal((P, D), dtype=np.float32) / math.sqrt(D),
        "bk": np.zeros(P, np.float32),
        "Wq": rng.standard_normal((P, D), dtype=np.float32) / math.sqrt(D),
        "bq": np.zeros(P, np.float32),
        "Wv": rng.standard_normal((D, D), dtype=np.float32) / math.sqrt(D),
        "bv": np.zeros(D, np.float32),
        "ln_g": np.ones(D, np.float32),
        "ln_b": np.zeros(D, np.float32),
        "Wo": rng.standard_normal((D, D), dtype=np.float32) / math.sqrt(D),
        "bo": np.zeros(D, np.float32),
    }
    out = kernel(**ins)
    print("out", out.shape, out.dtype, float(np.abs(out).mean()))


# revision 3
# speedup vs baseline: 76.3461x; 17.4057x over previous
"""PhasorLayer TRN2 kernel v2: data-parallel over batch, 8 NeuronCores.

Single-pass bf16 design (per batch row m):
  u     = x @ [Wk|Wq|wsum]^T + [bk|bq|sum_bv]          (KQS gemm, N=129, bf16)
  align = 64 - 2*sum_p sin^2((pi/2)*(tanh(uk)-tanh(uq)))
  gain  = softplus(align/64 + 0.5);  s = align*gain/64
  w     = x @ Wv^T + bv        (GEMM1, bf16, V^T kept resident in SBUF)
  muw   = mean(w);  varw = mean(w^2) - muw^2
  inv   = rsqrt(s^2*varw + 1e-5);  a = s*inv;  c = a*muw
  out   = xr + a*(w @ Wo'^T) - c*w1                    (GEMM2, bf16)
  where Wo' = Wo * ln_g (cols), w1 = rowsum(Wo'), xr = x + ln_b@Wo^T + bo
"""

import sys

sys.path.insert(0, "/opt/trn_rl_repo")

import math
import os
from contextlib import ExitStack

import ml_dtypes
import numpy as np

import concourse.bass as bass
import concourse.mybir as mybir
import concourse.tile as tile
from concourse.alu_op_type import AluOpType
from concourse.bass_utils import run_bass_kernel_spmd
from concourse.mybir import dt
from concourse.tile_cfg import (
    BassTileBranchHintPlaceholder,
    BassTileConditionalBlock,
    BassTileCriticalSection,
    BassTileLoopBlock,
    BassTileSwitchBlock,
    TileBranchInst,
)
from concourse.vector_clock import ScopedClock

B, D, P = 8192, 4096, 64
NCORES = 8
M = B // NCORES  # 1024 batch rows per core
MT = M // 128    # 8 m-tiles
KD = D // 128    # 32 dim tiles
NB = D // 512    # 8 n-blocks
PI = math.pi
EPS = 1e-5
F32 = dt.float32
BF16 = dt.bfloat16
NPBF16 = ml_dtypes.bfloat16
AF = mybir.ActivationFunctionType

_SKIP_SPLIT = (
    BassTileBranchHintPlaceholder,
    BassTileConditionalBlock,
    BassTileCriticalSection,
    BassTileLoopBlock,
    BassTileSwitchBlock,
    TileBranchInst,
)


class LegalTileContext(tile.TileContext):
    """TileContext legalized to <=1 semaphore wait per instruction.

    This container's walrus rejects instructions with >1 sync wait. Extra
    waits are peeled onto single-wait NoOps on the same engine.
    """

    def _lower_ordered_insts(self, ordered):
        for insts in ordered.values():
            out = []
            for inst in insts:
                si = getattr(inst, "sync_info", None)
                if (
                    si is not None
                    and len(si.on_wait) > 1
                    and not isinstance(inst, _SKIP_SPLIT)
                ):
                    waits = list(si.on_wait)
                    for w in waits[:-1]:
                        nop = mybir.InstNoOp(
                            name=self.nc.get_next_instruction_name(),
                            text_hint="wait_split",
                            bass_nofuse=True,
                            engine=inst.engine,
                            sync_info=mybir.SyncInfo(on_wait=[w], on_update=[]),
                        )
                        out.append(nop)
                    inst.sync_info = mybir.SyncInfo(
                        on_wait=[waits[-1]], on_update=list(si.on_update)
                    )
                out.append(inst)
            insts[:] = out
        super()._lower_ordered_insts(ordered)

    def _drain_and_barrier(self, tick_clock, wait_clock):
        drain_inst = self.nc.sync.drain()
        wait_clock.add_sem_waits(
            drain_inst.ins, ScopedClock({None: tick_clock.global_clock})
        )
        si = drain_inst.ins.sync_info
        if si is not None and len(si.on_wait) > 1:
            waits = list(si.on_wait)
            drain_inst.ins.sync_info = mybir.SyncInfo(
                on_wait=[waits[0]], on_update=list(si.on_update)
            )
            for w in waits[1:]:
                nop = self.nc.sync.nop(nofuse=True, hint="wait_split")
                nop.ins.sync_info = mybir.SyncInfo(on_wait=[w], on_update=[])
        self.nc.all_engine_barrier()
        assert self.sems is not None
        popped = self.nc._tile_sem_poison_stack.pop()
        assert popped is self._sem_poison
        self.nc.clear_and_free_semaphores(list(self.sems.allocated().values()))
        self.nc.all_engine_barrier()


def build_body(nc, tc, ctx, dram, rep):
    """Emit one full kernel execution under TileContext tc."""
    r = f"r{rep}_"
    (xt_d, wvb_d, wob_d, wkqsb_d, brow_d, bvr_d, w1m_d, xrm_d, out_d,
     ssq_dram) = dram

    sb_small = ctx.enter_context(tc.tile_pool(name=r + "small", bufs=1))

    ones_t = sb_small.tile((128, 1), BF16, name=r + "ones", tag=r + "ones")
    nc.vector.memset(ones_t[:], 1.0)
    half_t = sb_small.tile((128, 1), F32, name=r + "half", tag=r + "half")
    nc.vector.memset(half_t[:], 0.5)
    eps_t = sb_small.tile((128, 1), F32, name=r + "epsb", tag=r + "epsb")
    nc.vector.memset(eps_t[:], EPS)
    brow_t = sb_small.tile((128, 129), F32, name=r + "browt", tag=r + "browt")
    nc.gpsimd.dma_start(brow_t[:], brow_d[:, :])
    bvr_t = sb_small.tile((128, KD), F32, name=r + "bvrt", tag=r + "bvrt")
    nc.gpsimd.dma_start(bvr_t[:], bvr_d[:, :])

    def col_tile(nm):
        return sb_small.tile((128, MT), F32, name=r + nm, tag=r + nm)

    red_all = col_tile("red_all")
    align_all = col_tile("align_all")
    e1_all = col_tile("e1_all")
    gain_all = col_tile("gain_all")
    s2_all = col_tile("s2_all")
    mu_all = col_tile("mu_all")
    ssq_all = col_tile("ssq_all")
    musq_all = col_tile("musq_all")
    var_all = col_tile("var_all")
    s_all = col_tile("s_all")
    s_sq_all = col_tile("s_sq_all")
    q_all = col_tile("q_all")
    q2_all = col_tile("q2_all")
    inv_all = col_tile("inv_all")
    a_all = col_tile("a_all")
    c_all = col_tile("c_all")
    cneg_all = col_tile("cneg_all")
    acc_sb = sb_small.tile((1, M), F32, name=r + "acc_sb", tag=r + "acc_sb")
    sqw_last = sb_small.tile((128, M), BF16, name=r + "sqw_l", tag=r + "sqw_l")

    # wt residents persist across both phases
    sb_wt = ctx.enter_context(tc.tile_pool(name=r + "wtp", bufs=1))
    wt_res = [
        sb_wt.tile((128, M), BF16, name=f"{r}wtr{k}", tag=f"{r}wtr{k}")
        for k in range(KD)
    ]
    # prefetch buffer for GEMM2 nb=0 kd 0..7 (loaded mid-phase-1)
    wo_pre = sb_wt.tile((128, 8 * 512), BF16, name=r + "wo_pre", tag=r + "wo_pre")
    # ssq accumulator pool at outer scope: its final matmul + readout are
    # emitted in phase 2, after the first GEMM2 sweep
    ps_acc = ctx.enter_context(tc.tile_pool(name=r + "psacc", bufs=1, space="PSUM"))

    # ---------------- phase 1: KQS + GEMM1 (xt resident) ----------------
    with ExitStack() as p1:
        sb_xt = p1.enter_context(tc.tile_pool(name=r + "xtp", bufs=1))
        sb_s1 = p1.enter_context(tc.tile_pool(name=r + "s1", bufs=2))

        # bf16 x^T residents for GEMM1, produced on-chip from the fp32 KQS
        # stream (the phase path needs full fp32 accuracy: align is centered
        # near 0 and a = s*rsqrt(s^2 var + eps) amplifies bf16 noise ~300x
        # on near-zero-resonance rows).
        xt_res = [
            sb_xt.tile((128, M), BF16, name=f"{r}xt{j}", tag=f"{r}xt{j}")
            for j in range(KD)
        ]

        def xt_j(j):
            return xt_res[j][:]

        wkq_t = sb_xt.tile((128, KD * 129), F32, name=r + "wkq", tag=r + "wkq")
        WKC = KD * 129 // 4
        nc.scalar.dma_start(wkq_t[:, 0:WKC], wkqsb_d[:, 0:WKC])

        # KQS gemm in fp32: stationary = x^T m-slice, moving = wkqs[j]
        with ExitStack() as pk:
            ps_kq = pk.enter_context(tc.tile_pool(name=r + "pskq", bufs=1, space="PSUM"))
            kq_pair = [
                ps_kq.tile((128, 258), F32, name=f"{r}kqp{i}", tag=f"{r}kqp{i}")
                for i in range(MT // 2)
            ]
            kq_list = [
                kq_pair[t // 2][:, (t % 2) * 129 : (t % 2) * 129 + 129]
                for t in range(MT)
            ]
            for j in range(KD):
                if j in (1, 4, 7):
                    q = (j + 2) // 3
                    nc.scalar.dma_start(
                        wkq_t[:, q * WKC : (q + 1) * WKC],
                        wkqsb_d[:, q * WKC : (q + 1) * WKC],
                    )
                xs_t = sb_s1.tile((128, M), F32, name=r + "xs_t", tag=r + "xs", bufs=3)
                xs_eng = (nc.sync, nc.scalar, nc.gpsimd)[j % 3]
                xs_eng.dma_start(xs_t[:], xt_d[j * 128 : (j + 1) * 128, :])
                nc.scalar.activation(xt_res[j][:], xs_t[:], AF.Copy)
                for t in range(MT):
                    # two m-tiles share one PSUM bank => one accumulation
                    # group: start zeroes the bank at the first sub-tile,
                    # stop closes it at the last
                    nc.tensor.matmul(
                        kq_list[t],
                        xs_t[:, t * 128 : (t + 1) * 128],
                        wkq_t[:, j * 129 : (j + 1) * 129],
                        start=(j == 0 and t % 2 == 0),
                        stop=(j == KD - 1 and t % 2 == 1),
                    )
            # free all kq PSUM banks promptly: copy u = kq + brow to SBUF
            u_ts = []
            for t in range(MT):
                u_t = sb_s1.tile((128, 129), F32, name=r + "u_t", tag=r + "u", bufs=8)
                nc.vector.tensor_add(u_t[:], kq_list[t], brow_t[:])
                u_ts.append(u_t)

        nc.gpsimd.dma_start(wo_pre[:], wob_d[:, 0 : 8 * 512])

        # per-m-tile phase epilogue (ACT/DVE; overlaps GEMM1 matmuls on PE)
        for t in range(MT):
            u_t = u_ts[t]
            th_t = sb_s1.tile((128, 128), F32, name=r + "th_t", tag=r + "th")
            nc.scalar.activation(th_t[:], u_t[:, 0:128], AF.Tanh)
            d_t = sb_s1.tile((128, 64), F32, name=r + "d_t", tag=r + "d")
            nc.vector.tensor_sub(d_t[:], th_t[:, 0:64], th_t[:, 64:128])
            sn_t = sb_s1.tile((128, 64), F32, name=r + "sn_t", tag=r + "sn")
            nc.scalar.activation(sn_t[:], d_t[:], AF.Sin, scale=PI / 2)
            sq_t = sb_s1.tile((128, 64), F32, name=r + "sq_t", tag=r + "snsq")
            nc.scalar.activation(
                sq_t[:], sn_t[:], AF.Square, accum_out=red_all[:, t : t + 1]
            )
            nc.vector.tensor_scalar(
                align_all[:, t : t + 1],
                red_all[:, t : t + 1],
                -2.0,
                float(P),
                AluOpType.mult,
                AluOpType.add,
            )
            nc.scalar.activation(
                e1_all[:, t : t + 1],
                align_all[:, t : t + 1],
                AF.Exp,
                bias=half_t[:],
                scale=1.0 / P,
            )
            nc.scalar.activation(
                gain_all[:, t : t + 1], e1_all[:, t : t + 1], AF.Ln, bias=1.0
            )
            nc.vector.tensor_mul(
                s2_all[:, t : t + 1],
                align_all[:, t : t + 1],
                gain_all[:, t : t + 1],
            )
            nc.scalar.activation(
                mu_all[:, t : t + 1], u_t[:, 128:129], AF.Copy, scale=1.0 / D
            )

        # GEMM1: w^T tile kd = sum_j Wv^T[j, kd]^T @ x^T[j]  -> SBUF bf16
        # ssq = sum_k w^2 via ones-stationary matmuls, staggered one kd
        # behind the main stream so the PE never waits on sqw. The
        # accumulator pool lives at the outer scope: the final ssq matmul
        # and its readout are emitted in phase 2 (after the first GEMM2
        # sweep) so the mm->bias->square->ssq chain of kd=31 never stalls
        # the PE at the phase boundary.
        ps_v = p1.enter_context(tc.tile_pool(name=r + "psv", bufs=2, space="PSUM"))
        acc_ps0 = ps_acc.tile((1, 512), F32, name=r + "acc_ps0", tag=r + "acc0")
        acc_ps1 = ps_acc.tile((1, 512), F32, name=r + "acc_ps1", tag=r + "acc1")

        sqw_tiles = [None] * KD

        def emit_ssq(kd):
            sqw_t = sqw_tiles[kd]
            nc.tensor.matmul(
                acc_ps0[:], ones_t[:], sqw_t[:, 0:512],
                start=(kd == 0), stop=(kd == KD - 1),
            )
            nc.tensor.matmul(
                acc_ps1[:], ones_t[:], sqw_t[:, 512:1024],
                start=(kd == 0), stop=(kd == KD - 1),
            )

        for kd in range(KD):
            wv_t = sb_s1.tile((128, D), BF16, name=r + "wv_t", tag=r + "wv", bufs=3)
            wv_eng = (nc.sync, nc.scalar, nc.gpsimd)[kd % 3]
            wv_eng.dma_start(wv_t[:], wvb_d[:, kd * D : (kd + 1) * D])
            v_ps = [
                ps_v.tile((128, 512), F32, name=f"{r}v_ps{h}", tag=f"{r}vps{h}")
                for h in range(2)
            ]
            for b in range(KD):
                for h in range(2):
                    nc.tensor.matmul(
                        v_ps[h][:],
                        wv_t[:, b * 128 : (b + 1) * 128],
                        xt_j(b)[:, h * 512 : (h + 1) * 512],
                        start=(b == 0),
                        stop=(b == KD - 1),
                    )
            if kd >= 1:
                emit_ssq(kd - 1)
            # bias add + cast to bf16 resident; square for ssq
            for h in range(2):
                nc.vector.tensor_scalar(
                    wt_res[kd][:, h * 512 : (h + 1) * 512],
                    v_ps[h][:],
                    bvr_t[:, kd : kd + 1],
                    None,
                    AluOpType.add,
                )
            if kd == KD - 1:
                sqw_t = sqw_last
            else:
                sqw_t = sb_s1.tile(
                    (128, M), BF16, name=r + "sqw_t", tag=r + "sqw", bufs=2
                )
            nc.scalar.activation(sqw_t[:], wt_res[kd][:], AF.Square)
            sqw_tiles[kd] = sqw_t

    def emit_ssq_tail_and_finalize():
        emit_ssq(KD - 1)
        # ssq bounce: [1, M] -> DRAM -> [128, MT] columns
        nc.scalar.copy(acc_sb[:, 0:512], acc_ps0[:])
        nc.scalar.copy(acc_sb[:, 512:1024], acc_ps1[:])
        nc.scalar.dma_start(ssq_dram[:, :], acc_sb[:])
        for t in range(MT):
            nc.scalar.dma_start(
                ssq_all[:, t : t + 1],
                ssq_dram[0:1, t * 128 : (t + 1) * 128].transpose([1, 0]),
            )
        # scalar finalize
        nc.scalar.activation(musq_all[:], mu_all[:], AF.Square)
        nc.vector.tensor_scalar(var_all[:], ssq_all[:], 1.0 / D, None, AluOpType.mult)
        nc.vector.tensor_sub(var_all[:], var_all[:], musq_all[:])
        nc.scalar.activation(s_all[:], s2_all[:], AF.Copy, scale=1.0 / P)
        nc.scalar.activation(s_sq_all[:], s_all[:], AF.Square)
        nc.vector.tensor_mul(q_all[:], var_all[:], s_sq_all[:])
        nc.scalar.activation(q2_all[:], q_all[:], AF.Sqrt, bias=eps_t[:])
        nc.vector.reciprocal(inv_all[:], q2_all[:])
        nc.vector.tensor_mul(a_all[:], s_all[:], inv_all[:])
        nc.vector.tensor_mul(c_all[:], a_all[:], mu_all[:])
        nc.vector.tensor_scalar(cneg_all[:], c_all[:], -1.0, None, AluOpType.mult)

    # ---------------- phase 2: GEMM2 + epilogue (wt resident) ----------------
    with ExitStack() as p2:
        sb_s2 = p2.enter_context(tc.tile_pool(name=r + "s2", bufs=2))
        ps_p = p2.enter_context(tc.tile_pool(name=r + "psp", bufs=1, space="PSUM"))

        w1_res = sb_s2.tile((128, D), F32, name=r + "w1_res", tag=r + "w1_res", bufs=1)
        nc.sync.dma_start(w1_res[:], w1m_d[:, :])

        QN = 8 * 512  # wo quarter: 8 kd-slices of one nb

        def epilogue(nb, mt, p_tile):
            nsl = slice(nb * 512, (nb + 1) * 512)
            msl = slice(mt * 128, (mt + 1) * 128)
            xe_t = sb_s2.tile((128, 512), F32, name=r + "xe_t", tag=r + "xe", bufs=3)
            nc.scalar.dma_start(xe_t[:], xrm_d[msl, nsl])
            t1_t = sb_s2.tile((128, 512), F32, name=r + "t1_t", tag=r + "t1", bufs=3)
            nc.vector.scalar_tensor_tensor(
                t1_t[:], p_tile[:], a_all[:, mt : mt + 1], xe_t[:],
                AluOpType.mult, AluOpType.add,
            )
            oe_t = sb_s2.tile((128, 512), F32, name=r + "oe_t", tag=r + "oe", bufs=3)
            nc.vector.scalar_tensor_tensor(
                oe_t[:], w1_res[:, nsl], cneg_all[:, mt : mt + 1], t1_t[:],
                AluOpType.mult, AluOpType.add,
            )
            nc.sync.dma_start(out_d[msl, nsl], oe_t[:])

        # mt-major sweeps, epilogue interleaved per sweep; 6-bank PSUM ring
        # (acc_ps holds the other 2 banks until the deferred ssq readout).
        for nb in range(NB):
            # wo quarters for this nb (kd 0-7 of nb=0 comes from wo_pre)
            wo_q = []
            for q in range(4):
                if nb == 0 and q == 0:
                    wo_q.append(wo_pre)
                    continue
                t = sb_s2.tile((128, QN), BF16, name=r + "wo_q", tag=r + "woq", bufs=5)
                eng = (nc.gpsimd, nc.scalar)[(nb * 4 + q) % 2]
                eng.dma_start(
                    t[:],
                    wob_d[:, nb * (KD * 512) + q * QN : nb * (KD * 512) + (q + 1) * QN],
                )
                wo_q.append(t)

            for mt in range(MT):
                p_tile = ps_p.tile(
                    (128, 512), F32, name=f"{r}pp{mt % 6}", tag=f"{r}pp{mt % 6}"
                )
                for kd in range(KD):
                    wo_sl = wo_q[kd // 8][:, (kd % 8) * 512 : (kd % 8 + 1) * 512]
                    nc.tensor.matmul(
                        p_tile[:],
                        wt_res[kd][:, mt * 128 : (mt + 1) * 128],
                        wo_sl,
                        start=(kd == 0),
                        stop=(kd == KD - 1),
                    )
                if nb == 0 and mt == 0:
                    emit_ssq_tail_and_finalize()
                epilogue(nb, mt, p_tile)


def build_nc(reps=1):
    nc = bass.Bass()
    xt_d = nc.declare_dram_parameter("xt", [D, M], F32, isOutput=False)
    wvb_d = nc.declare_dram_parameter("wvb", [128, KD * D], BF16, isOutput=False)
    wob_d = nc.declare_dram_parameter("wob", [128, NB * KD * 512], BF16, isOutput=False)
    wkqsb_d = nc.declare_dram_parameter("wkqsb", [128, KD * 129], F32, isOutput=False)
    brow_d = nc.declare_dram_parameter("brow", [128, 129], F32, isOutput=False)
    bvr_d = nc.declare_dram_parameter("bvr", [128, KD], F32, isOutput=False)
    w1m_d = nc.declare_dram_parameter("w1m", [128, D], F32, isOutput=False)
    xrm_d = nc.declare_dram_parameter("xrm", [M, D], F32, isOutput=False)
    out_d = nc.declare_dram_parameter("out", [M, D], F32, isOutput=True)
    ssq_dram = nc.dram_tensor("ssq_scr", [1, M], F32)
    dram = (xt_d, wvb_d, wob_d, wkqsb_d, brow_d, bvr_d, w1m_d, xrm_d,
            out_d, ssq_dram)

    for rep in range(reps):
        with ExitStack() as ctx:
            tc = ctx.enter_context(LegalTileContext(nc))
            build_body(nc, tc, ctx, dram, rep)
    return nc


def prep_host(inputs):
    """Host-side weight layout prep shared by all cores."""
    Wk = np.asarray(inputs["Wk"], dtype=np.float32)
    bk = np.asarray(inputs["bk"], dtype=np.float32)
    Wq = np.asarray(inputs["Wq"], dtype=np.float32)
    bq = np.asarray(inputs["bq"], dtype=np.float32)
    Wv = np.asarray(inputs["Wv"], dtype=np.float32)
    bv = np.asarray(inputs["bv"], dtype=np.float32)
    ln_g = np.asarray(inputs["ln_g"], dtype=np.float32)
    ln_b = np.asarray(inputs["ln_b"], dtype=np.float32)
    Wo = np.asarray(inputs["Wo"], dtype=np.float32)
    bo = np.asarray(inputs["bo"], dtype=np.float32)

    Wo2T = np.ascontiguousarray((Wo * ln_g[None, :]).T)  # [k, n]
    w1 = Wo2T.sum(axis=0)  # [n]
    rrow = (ln_b @ Wo.T + bo).astype(np.float32)  # [n]
    wsum = Wv.sum(axis=0)  # [j]
    wkqs = np.concatenate([Wk.T, Wq.T, wsum[:, None]], axis=1).astype(np.float32)
    brow = np.concatenate([bk, bq, [bv.sum()]]).astype(np.float32)

    # [128, KD*D]: wvb[p, kd*D + b*128 + kk] = Wv[kd*128+kk, b*128+p]
    wvb = np.ascontiguousarray(
        Wv.reshape(KD, 128, KD, 128).transpose(3, 0, 2, 1).reshape(128, KD * D)
    ).astype(NPBF16)
    # [128, NB*KD*512]: wob[p, nb*KD*512 + kd*512 + n'] = Wo2T[kd*128+p, nb*512+n']
    wob = np.ascontiguousarray(
        Wo2T.reshape(KD, 128, NB, 512).transpose(1, 2, 0, 3).reshape(128, NB * KD * 512)
    ).astype(NPBF16)
    # [128, KD*129]: wkqsb[p, j*129+c] = wkqs[j*128+p, c]  (fp32)
    wkqsb = np.ascontiguousarray(
        wkqs.reshape(KD, 128, 129).transpose(1, 0, 2).reshape(128, KD * 129)
    )
    brow_mat = np.ascontiguousarray(np.broadcast_to(brow, (128, 129)))
    bvr = np.ascontiguousarray(bv.reshape(KD, 128).T)  # [128, KD]
    w1m = np.ascontiguousarray(np.broadcast_to(w1, (128, D)), dtype=np.float32)
    return wvb, wob, wkqsb, brow_mat, bvr, w1m, rrow


def make_in_maps(x, prep):
    wvb, wob, wkqsb, brow_mat, bvr, w1m, rrow = prep
    in_maps = []
    for c in range(NCORES):
        xc = np.ascontiguousarray(x[c * M : (c + 1) * M])
        xtc = np.ascontiguousarray(xc.T)
        xrm = np.ascontiguousarray(xc + rrow[None, :])
        in_maps.append(
            {
                "xt": xtc,
                "wvb": wvb,
                "wob": wob,
                "wkqsb": wkqsb,
                "brow": brow_mat,
                "bvr": bvr,
                "w1m": w1m,
                "xrm": xrm,
            }
        )
    return in_maps


def kernel(**inputs):
    x = np.asarray(inputs["x"], dtype=np.float32)
    prep = prep_host(inputs)
    nc = build_nc(reps=1)
    in_maps = make_in_maps(x, prep)
    global LAST_BUILD
    LAST_BUILD = (nc, in_maps)
    res = run_bass_kernel_spmd(nc, in_maps, core_ids=list(range(NCORES)))
    global LAST_EXEC_NS
    LAST_EXEC_NS = getattr(res, "exec_time_ns", None)
    out = np.concatenate([res.results[c]["out"] for c in range(NCORES)], axis=0)
    return out.astype(np.float32)


LAST_EXEC_NS = None
LAST_BUILD = None


if __name__ == "__main__":
    rng = np.random.default_rng(0)
    ins = {
        "x": rng.standard_normal((B, D), dtype=np.float32),
        "Wk": rng.standard_normal((P, D), dtype=np.float32) / math.sqrt(D),
        "bk": np.zeros(P, np.float32),
        "Wq": rng.standard_normal((P, D), dtype=np.float32) / math.sqrt(D),
        "bq": np.zeros(P, np.float32),
        "Wv": rng.standard_normal((D, D), dtype=np.float32) / math.sqrt(D),
        "bv": np.zeros(D, np.float32),
        "ln_g": np.ones(D, np.float32),
        "ln_b": np.zeros(D, np.float32),
        "Wo": rng.standard_normal((D, D), dtype=np.float32) / math.sqrt(D),
        "bo": np.zeros(D, np.float32),
    }
    out = kernel(**ins)
    print("out", out.shape, out.dtype, float(np.abs(out).mean()))
